# revision 1
# baseline (speedup 1.0000x reference)
"""Causal self-attention block (B=4, T=2048, C=2048, H=16, D=128) on 8 trn2 cores.

Sharding: tensor-parallel over head groups (2 groups of 8 heads) x
data-parallel over batch (4). Core (g, b) computes, for batch b and heads
[8g, 8g+8): qkv projection, causal attention, and the partial output
projection contribution attn_out[:, heads_g] @ Wproj[rows_g]. The host sums
the two partial yT per batch, adds bproj, and transposes back.

All matmuls run as float32r (fp32 rounded to 12-bit mantissa; exact on
pre-rounded inputs at full PE speed). Everything on-chip is kept in
transposed layouts so no fp32 DMA-transposes are needed:
  phase 1: qkvT[col, t] = W^T x^T  (24 col-tiles of 128, t in chunks of 512)
  phase 2: per head: S^T = K^T^T Q^T blocks -> exp -> causal mask; column
           sums via ones-matmul; out^T = V^T P^T; normalize by 1/colsum.
           V (natural layout, AV lhsT) comes from 128x128 PE transposes of V^T.
  phase 3: yT = Wproj_g^T attn_outT  (accumulate over the 8 head chunks)
"""

import sys

sys.path.insert(0, "/opt/trn_rl_repo")

import numpy as np

import concourse.bass as bass
import concourse.mybir as mybir
import concourse.tile as tile
from concourse import bacc
from concourse.bass_utils import run_bass_kernel_spmd
from concourse.masks import make_identity

F32 = mybir.dt.float32
F32R = mybir.dt.float32r
AF = mybir.ActivationFunctionType

B, T, C = 4, 2048, 2048
H, D = 16, 128
G = 2  # head-group shards
HPC = H // G  # heads per core = 8
CT = C // 128  # contraction chunks = 16
NT = T // 512  # t chunks of 512 = 4
NJ = 3 * HPC  # qkv col tiles per core = 24
SCALE = 1.0 / float(np.sqrt(D))
SUMS_ON_DVE = False  # accumulate softmax denominators on DVE, 1 ones-MM/chunk


def round_fp32r(x: np.ndarray) -> np.ndarray:
    """Round fp32 to fp32r (low 12 mantissa bits, round-to-nearest-even)."""
    u = np.ascontiguousarray(x, dtype=np.float32).view(np.uint32).astype(np.uint64)
    r = (u + 0x7FF + ((u >> 12) & 1)) & ~np.uint64(0xFFF)
    return r.astype(np.uint32).view(np.float32).reshape(x.shape)


def build_nc(phases=(1, 2, 3), reps=1):
    nc = bacc.Bacc("TRN2", target_bir_lowering=False)
    xT = nc.dram_tensor("xT", [128, CT, T], F32R, kind="ExternalInput")
    wqkv = nc.dram_tensor("wqkv", [128, NJ, CT, 128], F32R, kind="ExternalInput")
    wproj = nc.dram_tensor("wproj", [128, CT, HPC, 128], F32R, kind="ExternalInput")
    bqkv = nc.dram_tensor("bqkv", [128, NJ], F32, kind="ExternalInput")
    masks = nc.dram_tensor("masks", [128, 4, 512], F32R, kind="ExternalInput")
    yT = nc.dram_tensor("yT", [C, T], F32, kind="ExternalOutput")
    yT_r = yT.rearrange("(i p) t -> p i t", p=128)

    with tile.TileContext(nc) as tc:
        with (
            tc.tile_pool(name="const", bufs=1) as cst,
            tc.tile_pool(name="dram", bufs=1, space="DRAM") as dram,
        ):
            # allocate constants up front; their loads are emitted after the
            # warm-start DMAs so the first matmul's deps go first in the queue
            tri_sb = cst.tile([128, 128], F32R)
            bias_sb = cst.tile([128, NJ], F32)
            ident = cst.tile([128, 128], F32)
            ones_f = cst.tile([128, 2], F32)
            ones = cst.tile([128, 2], F32R)
            zf = cst.tile([128, 128], F32)
            zr = cst.tile([128, 128], F32R)

            def load_consts():
                nc.sync.dma_start(tri_sb, masks[:, 0, 0:128])
                nc.sync.dma_start(bias_sb, bqkv.ap())
                make_identity(nc, ident)
                nc.vector.memset(ones_f, 1.0)
                nc.vector.tensor_copy(ones, ones_f)
                nc.vector.memset(zf, 0.0)
                nc.vector.tensor_copy(zr, zf)

            qkvT = [
                dram.tile([128, T], F32R, name=f"qkvT{j}", tag=f"qkvT{j}")
                for j in range(NJ)
            ]

            if 1 not in phases:
                load_consts()

            for _rep in range(reps):
                # ---------------- phase 1: qkvT[col, t] = W^T x^T (+bias) -------
                if 1 in phases:
                 with (
                    tc.tile_pool(name=f"p1x_{_rep}", bufs=1) as p1x,
                    tc.tile_pool(name=f"p1w_{_rep}", bufs=3) as p1w,
                    tc.tile_pool(name=f"p1s_{_rep}", bufs=4) as p1s,
                    tc.tile_pool(name=f"ps1_{_rep}", bufs=8, space="PSUM") as ps1,
                ):
                    # interleave q/k/v col-tiles so head h's three tensors are all
                    # ready after 3*(h+1) of the 24 tiles
                    j_order = [base + h for h in range(HPC) for base in (0, HPC, 2 * HPC)]
                    WARM = 2  # first j's run chunk-outer to overlap the xs load
                    warm_w = {}
                    for wj in j_order[:WARM]:
                        w_sb = p1w.tile([128, CT, 128], F32R, tag="w")
                        nc.sync.dma_start(w_sb, wqkv[:, wj])
                        warm_w[wj] = w_sb
                    xs = p1x.tile([128, CT, T], F32R)
                    for cc in range(CT):
                        nc.sync.dma_start(xs[:, cc, :], xT[:, cc, :])
                    if _rep == 0:
                        load_consts()
                    # warm-up: 8 psum groups accumulate chunk-by-chunk as the xs
                    # chunks arrive, so PE works during the 16MB load
                    warm_ps = {
                        (wj, c): ps1.tile(
                            [128, 512], F32, tag="ps", name=f"warm_ps_{wj}_{c}"
                        )
                        for wj in j_order[:WARM]
                        for c in range(NT)
                    }
                    for cc in range(CT):
                        for wj in j_order[:WARM]:
                            for c in range(NT):
                                nc.tensor.matmul(
                                    warm_ps[(wj, c)],
                                    lhsT=warm_w[wj][:, cc, :],
                                    rhs=xs[:, cc, 512 * c : 512 * (c + 1)],
                                    start=(cc == 0),
                                    stop=(cc == CT - 1),
                                )
                    for wj in j_order[:WARM]:
                        for c in range(NT):
                            st = p1s.tile([128, 512], F32R, tag="st")
                            nc.vector.tensor_scalar_add(
                                st, warm_ps[(wj, c)], bias_sb[:, wj : wj + 1]
                            )
                            nc.sync.dma_start(qkvT[wj][:, 512 * c : 512 * (c + 1)], st)
                    for j in j_order[WARM:]:
                        w_sb = p1w.tile([128, CT, 128], F32R, tag="w")
                        nc.sync.dma_start(w_sb, wqkv[:, j])
                        for c in range(NT):
                            ps = ps1.tile([128, 512], F32, tag="ps")
                            for cc in range(CT):
                                nc.tensor.matmul(
                                    ps,
                                    lhsT=w_sb[:, cc, :],
                                    rhs=xs[:, cc, 512 * c : 512 * (c + 1)],
                                    start=(cc == 0),
                                    stop=(cc == CT - 1),
                                )
                            st = p1s.tile([128, 512], F32R, tag="st")
                            nc.vector.tensor_scalar_add(st, ps, bias_sb[:, j : j + 1])
                            nc.sync.dma_start(qkvT[j][:, 512 * c : 512 * (c + 1)], st)

                # ---- phases 2+3 share a persistent SBUF pool holding the
                # attention outputs (no DRAM round-trip, no phase-3 reload) ----
                if 2 in phases:
                 with tc.tile_pool(name=f"otp_{_rep}", bufs=1) as otp:
                  ot_tiles = {}
                  with (
                    tc.tile_pool(name=f"p2qk_{_rep}", bufs=2) as p2qk,
                    tc.tile_pool(name=f"p2v_{_rep}", bufs=2) as p2v,
                    tc.tile_pool(name=f"p2p_{_rep}", bufs=3) as p2p,
                    tc.tile_pool(name=f"p2sc_{_rep}", bufs=4) as p2sc,
                    tc.tile_pool(name=f"ps2s_{_rep}", bufs=3, space="PSUM") as ps2s,
                    tc.tile_pool(name=f"ps2t_{_rep}", bufs=2, space="PSUM") as ps2t,
                    tc.tile_pool(name=f"ps2m_{_rep}", bufs=1, space="PSUM") as ps2m,
                    tc.tile_pool(name=f"ps2o_{_rep}", bufs=2, space="PSUM") as ps2o,
                    tc.tile_pool(name=f"dram_rb_{_rep}", bufs=4, space="DRAM") as dram_rb,
                  ):
                    for h in range(HPC):
                        q_sb = p2qk.tile([128, T], F32R, tag="q")
                        nc.sync.dma_start(q_sb, qkvT[h][:])
                        k_sb = p2qk.tile([128, T], F32R, tag="k")
                        nc.sync.dma_start(k_sb, qkvT[HPC + h][:])
                        vt_sb = p2qk.tile([128, T], F32R, tag="vt")
                        nc.sync.dma_start(vt_sb, qkvT[2 * HPC + h][:])

                        # V natural layout via PE transposes of V^T 128x128 blocks
                        v_sb = p2v.tile([128, T // 128, 128], F32R, tag="v")
                        for j in range(T // 128):
                            ps_v = ps2t.tile([128, 128], F32, tag="pst")
                            nc.tensor.transpose(
                                ps_v, vt_sb[:, 128 * j : 128 * (j + 1)].bitcast(F32), ident
                            )
                            nc.vector.tensor_copy(v_sb[:, j, :], ps_v)

                        tri = tri_sb
                        for c in range(NT):
                            nblk = 4 * c + 4
                            pta = p2p.tile(
                                [128, 8, 512], F32R, tag="pT", name=f"pta_{h}_{c}"
                            )
                            ptb = (
                                p2p.tile(
                                    [128, 8, 512], F32R, tag="pT", name=f"ptb_{h}_{c}"
                                )
                                if nblk > 8
                                else None
                            )

                            def pT(j):
                                return (pta if j < 8 else ptb)[:, j % 8, :]

                            acc = None
                            for j in range(nblk):
                                v = j - 4 * c  # >= 0 on diagonal-group blocks
                                off = 128 * v if v > 0 else 0
                                ps_s = ps2s.tile([128, 512], F32, tag="s")
                                # diag blocks: compute only a suffix covering
                                # the valid columns, kept >=256 wide so fp32r
                                # stays at 1 cyc/row (v=3's 128-wide valid
                                # region is computed as a 256-wide slice)
                                moff = min(off, 256) if v > 0 else 0
                                nc.tensor.matmul(
                                    ps_s[:, moff:512],
                                    lhsT=k_sb[:, 128 * j : 128 * (j + 1)],
                                    rhs=q_sb[:, 512 * c + moff : 512 * (c + 1)],
                                    start=True,
                                    stop=True,
                                )
                                # exp only over the causally-reachable columns;
                                # columns < off are never read downstream.
                                nc.scalar.activation(
                                    pT(j)[:, off:512], ps_s[:, off:512], AF.Exp,
                                    scale=SCALE,
                                )
                                if v >= 0:
                                    nc.vector.tensor_mul(
                                        pT(j)[:, off : off + 128],
                                        pT(j)[:, off : off + 128],
                                        tri,
                                    )
                                if v == 3:
                                    # zero-fill so sum/AV can read a 256-wide
                                    # slice (N=256 keeps fp32r at 1 cyc/row)
                                    nc.vector.tensor_copy(
                                        pT(j)[:, 256:384], zr
                                    )
                                if SUMS_ON_DVE:
                                    if j == 0:
                                        acc = p2sc.tile(
                                            [128, 512], F32R, tag="acc",
                                            name=f"acc_{h}_{c}",
                                        )
                                        nc.vector.tensor_copy(acc, pT(0))
                                    else:
                                        nc.vector.tensor_add(
                                            acc[:, off:512],
                                            acc[:, off:512],
                                            pT(j)[:, off:512],
                                        )
                            ps_sum = ps2m.tile([2, 512], F32, tag="sum")
                            if SUMS_ON_DVE:
                                nc.tensor.matmul(
                                    ps_sum, lhsT=ones, rhs=acc, start=True, stop=True
                                )
                            else:
                                for j in range(nblk):
                                    v = j - 4 * c
                                    off = 128 * v if v > 0 else 0
                                    nc.tensor.matmul(
                                        ps_sum[:, off:512],
                                        lhsT=ones,
                                        rhs=pT(j)[:, off:512],
                                        start=(j == 0),
                                        stop=(j == nblk - 1),
                                    )
                            rs = p2sc.tile([1, 512], F32, tag="rs")
                            nc.vector.reciprocal(rs, ps_sum[0:1, :])
                            rbx = dram_rb.tile([1, 512], F32, tag="rbx")
                            nc.sync.dma_start(rbx, rs)
                            rb = p2sc.tile([128, 512], F32, tag="rb")
                            nc.gpsimd.dma_start(rb, rbx[0].partition_broadcast(128))
                            ps_o = ps2o.tile([128, 512], F32, tag="o")
                            for j in range(nblk):
                                v = j - 4 * c
                                off = min(128 * v, 256) if v > 0 else 0
                                nc.tensor.matmul(
                                    ps_o[:, off:512],
                                    lhsT=v_sb[:, j, :],
                                    rhs=pT(j)[:, off:512],
                                    start=(j == 0),
                                    stop=(j == nblk - 1),
                                )
                            ot = otp.tile(
                                [128, 512], F32R, name=f"ot_{h}_{c}", tag=f"ot_{h}_{c}"
                            )
                            nc.vector.tensor_mul(ot, ps_o, rb)
                            ot_tiles[(h, c)] = ot

                  # -------- phase 3: yT = Wproj_g^T attn_outT (from SBUF) -------
                  if 3 in phases:
                   with (
                      tc.tile_pool(name=f"p3w_{_rep}", bufs=3) as p3w,
                      tc.tile_pool(name=f"p3y_{_rep}", bufs=4) as p3y,
                      tc.tile_pool(name=f"ps3_{_rep}", bufs=4, space="PSUM") as ps3,
                   ):
                    for i in range(CT):
                        wp = p3w.tile([128, HPC, 128], F32R, tag="wp")
                        nc.sync.dma_start(wp, wproj[:, i])
                        for c in range(NT):
                            ps_y = ps3.tile([128, 512], F32, tag="y")
                            for hh in range(HPC):
                                nc.tensor.matmul(
                                    ps_y,
                                    lhsT=wp[:, hh, :],
                                    rhs=ot_tiles[(hh, c)][:],
                                    start=(hh == 0),
                                    stop=(hh == HPC - 1),
                                )
                            ys = p3y.tile([128, 512], F32, tag="ys")
                            nc.vector.tensor_copy(ys, ps_y)
                            nc.sync.dma_start(yT_r[:, i, 512 * c : 512 * (c + 1)], ys)

    nc.compile()
    return nc


_NC_CACHE = None


def _get_nc():
    global _NC_CACHE
    if _NC_CACHE is None:
        _NC_CACHE = build_nc()
    return _NC_CACHE


def _prep_inputs(x, Wqkv, bqkv, Wproj):
    """Host-side shard + pre-tile + fp32r-round. Returns list of 8 in_maps,
    core index = g * B + b."""
    x = round_fp32r(np.asarray(x))
    Wqkv = round_fp32r(np.asarray(Wqkv))
    Wproj = round_fp32r(np.asarray(Wproj))
    bqkv = np.asarray(bqkv, dtype=np.float32)

    # causal mask variants for the diagonal 512-chunks
    p = np.arange(128)[:, None]
    f = np.arange(512)[None, :]
    masks = np.stack(
        [(f >= 128 * v + p).astype(np.float32) for v in range(4)], axis=1
    )  # [128, 4, 512]
    masks = np.ascontiguousarray(masks)

    # xT tiles per batch: [128, CT, T] with [p, o, t] = x[b, t, o*128+p]
    xT_b = []
    for b in range(B):
        xt = np.ascontiguousarray(x[b].T)  # [C, T]
        xT_b.append(np.ascontiguousarray(xt.reshape(CT, 128, T).transpose(1, 0, 2)))

    in_maps = [None] * (G * B)
    for g in range(G):
        cols = np.concatenate(
            [
                np.arange(g * 1024, (g + 1) * 1024),
                np.arange(C + g * 1024, C + (g + 1) * 1024),
                np.arange(2 * C + g * 1024, 2 * C + (g + 1) * 1024),
            ]
        )
        wg = Wqkv[:, cols]  # [C, 3072] = [(o p), (j m)]
        # -> [128 p, 24 j, 16 o, 128 m]
        wg_t = np.ascontiguousarray(
            wg.reshape(CT, 128, NJ, 128).transpose(1, 2, 0, 3)
        )
        bg = bqkv[cols]  # [3072]
        bg_t = np.ascontiguousarray(bg.reshape(NJ, 128).T)  # [128, 24]
        wp = Wproj[g * 1024 : (g + 1) * 1024, :]  # [1024, C] = [(h p), (i m)]
        # -> [128 p, 16 i, 8 h, 128 m]
        wp_t = np.ascontiguousarray(
            wp.reshape(HPC, 128, CT, 128).transpose(1, 2, 0, 3)
        )
        for b in range(B):
            in_maps[g * B + b] = dict(
                xT=xT_b[b], wqkv=wg_t, wproj=wp_t, bqkv=bg_t, masks=masks
            )
    return in_maps


def kernel(x, Wqkv, bqkv, Wproj, bproj):
    x = np.asarray(x)
    nc = _get_nc()
    in_maps = _prep_inputs(x, Wqkv, bqkv, Wproj)
    res = run_bass_kernel_spmd(nc, in_maps, core_ids=list(range(G * B)))
    y = np.empty((B, T, C), dtype=np.float32)
    bp = np.asarray(bproj, dtype=np.float32)
    for b in range(B):
        acc = res.results[b]["yT"].astype(np.float32).copy()
        for g in range(1, G):
            acc += res.results[g * B + b]["yT"]
        y[b] = acc.T + bp[None, :]
    return y



# revision 3
# speedup vs baseline: 78275.9990x; 78275.9990x over previous
"""Causal self-attention (B=4, T=2048, C=2048, H=16, D=128) on 8 trn2 cores.

Sharding: tensor-parallel over head groups (2 groups of 8 heads) x
data-parallel over batch (4). Core (g, b) computes, for batch b and heads
[8g, 8g+8): qkv projection, causal attention, and the partial output
projection yT = Wproj_g^T attn_outT. The host sums the two partial yT per
batch, adds bproj, and transposes back.

All PE inputs are fp16 (1 cyc/row at any output width; psum accumulates
fp32). The whole kernel is one fused pass: q/k stay in SBUF (no DRAM
round trip), V is produced in natural [t, d] layout directly by phase 1
(lhsT = x^T tiles), softmax denominators accumulate on DVE with a single
ones-matmul per 512-chunk, and the reciprocal row broadcast runs on
GpSimd. Attention outputs spill to DRAM (small) and stream back for the
output projection. Emission is software-pipelined: head h's qkv
projection interleaves with head h-1's attention, and the output
projection interleaves with head 7's attention, so PE never idles on
Act/DVE.
"""

import itertools
import sys

sys.path.insert(0, "/opt/trn_rl_repo")

import numpy as np

import concourse.bass as bass
import concourse.mybir as mybir
import concourse.tile as tile
from concourse import bacc
from concourse.bass_utils import run_bass_kernel_spmd

F32 = mybir.dt.float32
F16 = mybir.dt.float16
AF = mybir.ActivationFunctionType

B, T, C = 4, 2048, 2048
H, D = 16, 128
G = 2  # head-group shards
HPC = H // G  # heads per core = 8
CT = C // 128  # contraction chunks = 16
NT = T // 512  # t chunks of 512 = 4
TB = T // 128  # t blocks of 128 = 16
SCALE = 1.0 / float(np.sqrt(D))


def _pump(gen):
    try:
        next(gen)
        return True
    except StopIteration:
        return False


def _drive(pri, sec, ratio):
    """Emit all of `sec`, inserting ~`ratio` steps of `pri` after each step.
    `sec` paces (latency-sensitive PE ops); `pri` is filler. Leftover `pri`
    drains at the end."""
    debt = 0.0
    pri_alive = True
    while _pump(sec):
        debt += ratio
        while debt >= 1.0 and pri_alive:
            pri_alive = _pump(pri)
            debt -= 1.0
    while pri_alive:
        pri_alive = _pump(pri)


def build_nc(reps=1, vbias=True):
    nc = bacc.Bacc("TRN2", target_bir_lowering=False)
    xT = nc.dram_tensor("xT", [128, CT, T], F16, kind="ExternalInput")
    wqk = nc.dram_tensor("wqk", [128, HPC, 2, CT, 128], F16, kind="ExternalInput")
    wv = nc.dram_tensor("wv", [128, CT, 2, 512], F16, kind="ExternalInput")
    wproj = nc.dram_tensor("wproj", [128, CT, HPC, 128], F16, kind="ExternalInput")
    bqk = nc.dram_tensor("bqk", [128, 2 * HPC], F32, kind="ExternalInput")
    bv = nc.dram_tensor("bv", [1, 2, 512], F16, kind="ExternalInput")
    tri_d = nc.dram_tensor("tri", [128, 128], F16, kind="ExternalInput")
    yT = nc.dram_tensor("yT", [C, T], F16, kind="ExternalOutput")
    yT_r = yT.rearrange("(i p) t -> p i t", p=128)

    with tile.TileContext(nc) as tc:
        with tc.tile_pool(name="const", bufs=1) as cst:
            tri = cst.tile([128, 128], F16)
            bias_sb = cst.tile([128, 2 * HPC], F32)
            bv_sb = cst.tile([1, 2, 512], F16)
            ones1 = cst.tile([1, 128], F16)  # ones row: rank-1 bias lhsT
            onesc = cst.tile([128, 1], F16)  # ones col: denominator lhsT

            def load_consts():
                nc.sync.dma_start(tri, tri_d.ap())
                nc.sync.dma_start(bias_sb, bqk.ap())
                nc.sync.dma_start(bv_sb, bv.ap())
                nc.vector.memset(ones1, 1.0)
                nc.vector.memset(onesc, 1.0)

            for _rep in range(reps):
                with (
                    tc.tile_pool(name=f"qkp_{_rep}", bufs=4) as qkp,
                    tc.tile_pool(name=f"vsb_{_rep}", bufs=2) as vsb,
                    tc.tile_pool(name=f"ppa_{_rep}", bufs=2) as ppa,
                    tc.tile_pool(name=f"ppb_{_rep}", bufs=1) as ppb,
                    tc.tile_pool(name=f"accp_{_rep}", bufs=2) as accp,
                    tc.tile_pool(name=f"rsp_{_rep}", bufs=2) as rsp,
                    tc.tile_pool(name=f"otp_{_rep}", bufs=3) as otp,
                    tc.tile_pool(name=f"otrp_{_rep}", bufs=1) as otrp,
                    tc.tile_pool(name=f"ps_p1_{_rep}", bufs=2, space="PSUM") as ps_p1,
                    tc.tile_pool(name=f"ps_sc_{_rep}", bufs=3, space="PSUM") as ps_sc,
                    tc.tile_pool(name=f"ps_av_{_rep}", bufs=2, space="PSUM") as ps_av,
                    tc.tile_pool(name=f"ps_ms_{_rep}", bufs=1, space="PSUM") as ps_ms,
                    tc.tile_pool(name=f"otd_{_rep}", bufs=1, space="DRAM") as otdp,
                ):
                    qk_tiles = {}  # h -> (q_sb, k_sb)
                    v_tiles = {}  # u -> v_sb tile [128, TB, 4, 128]
                    ot_res = {}  # (h, c) -> resident ot tile (heads 4-7)
                    otd = otdp.tile([128, NT, 4, 512], F16, name=f"otd_{_rep}")
                    # deferred chunk finisher: the denominator matmul +
                    # normalize of chunk c are emitted one chunk later so the
                    # DVE add-chain never gates the PE queue
                    pend = [None]

                    def finish_pend():
                        h0, c0, acc0, ps_o0 = pend[0]
                        pend[0] = None
                        ps_sum = ps_ms.tile([1, 512], F32, tag="m")
                        nc.tensor.matmul(
                            ps_sum, lhsT=onesc, rhs=acc0, start=True, stop=True
                        )
                        yield
                        rs = rsp.tile([1, 512], F32, tag="rs")
                        nc.vector.reciprocal(rs, ps_sum)
                        rb = rsp.tile([128, 512], F32, tag="rb")
                        nc.gpsimd.partition_broadcast(rb, rs)
                        if h0 < 4:
                            ot = otp.tile([128, 512], F16, tag="ot")
                            nc.vector.tensor_mul(ot, ps_o0, rb)
                            nc.gpsimd.dma_start(otd[:, c0, h0, :], ot)
                        else:
                            ot = otrp.tile(
                                [128, 512], F16, tag=f"otr_{h0}_{c0}",
                                name=f"otr_{_rep}_{h0}_{c0}",
                            )
                            nc.vector.tensor_mul(ot, ps_o0, rb)
                            ot_res[(h0, c0)] = ot

                    def gen_p1_head(h, w_sb, xs):
                        """q/k projection for head h: qT/kT = Wqk^T x^T."""
                        q_sb = qkp.tile([128, T], F16, tag="qk", name=f"q_{_rep}_{h}")
                        k_sb = qkp.tile([128, T], F16, tag="qk", name=f"k_{_rep}_{h}")
                        qk_tiles[h] = (q_sb, k_sb)
                        for s in range(2):
                            dst = (q_sb, k_sb)[s]
                            for c in range(NT):
                                ps = ps_p1.tile([128, 512], F32, tag="p1")
                                for cc in range(CT):
                                    nc.tensor.matmul(
                                        ps,
                                        lhsT=w_sb[:, s, cc, :],
                                        rhs=xs[:, cc, 512 * c : 512 * (c + 1)],
                                        start=(cc == 0),
                                        stop=(cc == CT - 1),
                                    )
                                    yield
                                nc.vector.tensor_scalar_add(
                                    dst[:, 512 * c : 512 * (c + 1)],
                                    ps,
                                    bias_sb[:, s * HPC + h : s * HPC + h + 1],
                                )

                    def gen_V(u, wv_sb, xs):
                        """V for 4-head subgroup u in natural [t, d] layout:
                        V = x Wv (+ rank-1 bias)."""
                        vt = vsb.tile(
                            [128, TB, 4, 128], F16, tag="v", name=f"v_{_rep}_{u}"
                        )
                        v_tiles[u] = vt
                        for tb in range(TB):
                            ps = ps_p1.tile([128, 512], F32, tag="p1")
                            for cc in range(CT):
                                nc.tensor.matmul(
                                    ps,
                                    lhsT=xs[:, cc, 128 * tb : 128 * (tb + 1)],
                                    rhs=wv_sb[:, cc, :],
                                    start=(cc == 0),
                                    stop=(cc == CT - 1 and not vbias),
                                )
                                yield
                            if vbias:
                                nc.tensor.matmul(
                                    ps, lhsT=ones1, rhs=bv_sb[:, u, :],
                                    start=False, stop=True,
                                )
                                yield
                            nc.vector.tensor_copy(vt[:, tb, :, :], ps)

                    def gen_p2_chunk(h, c):
                        """Attention for head h, query chunk c."""
                        q_sb, k_sb = qk_tiles[h]
                        vt = v_tiles[h // 4]
                        hh = h % 4
                        nblk = 4 * c + 4
                        pa = ppa.tile(
                            [128, 8, 512], F16, tag="P", name=f"pa_{_rep}_{h}_{c}"
                        )
                        pb = (
                            ppb.tile(
                                [128, 8, 512], F16, tag="P", name=f"pb_{_rep}_{h}_{c}"
                            )
                            if nblk > 8
                            else None
                        )

                        def P(j):
                            return (pa if j < 8 else pb)[:, j % 8, :]

                        def off_of(j):
                            v = j - 4 * c
                            return 128 * v if v > 0 else 0

                        for j in range(nblk):
                            off = off_of(j)
                            ps_s = ps_sc.tile([128, 512], F32, tag="s")
                            nc.tensor.matmul(
                                ps_s[:, off:512],
                                lhsT=k_sb[:, 128 * j : 128 * (j + 1)],
                                rhs=q_sb[:, 512 * c + off : 512 * (c + 1)],
                                start=True,
                                stop=True,
                            )
                            yield
                            nc.scalar.activation(
                                P(j)[:, off:512], ps_s[:, off:512], AF.Exp, scale=SCALE
                            )
                            if j - 4 * c >= 0:
                                nc.vector.tensor_mul(
                                    P(j)[:, off : off + 128],
                                    P(j)[:, off : off + 128],
                                    tri,
                                )
                        # denominators: accumulate on DVE, then one ones-matmul.
                        # first add fuses the init copy: acc = P0 + P1 where
                        # P1 is full-width (c>=1); c==0 keeps copy-then-add
                        acc = accp.tile([128, 512], F16, tag="acc")
                        if c == 0:
                            nc.vector.tensor_copy(acc, P(0))
                            j0 = 1
                        else:
                            nc.vector.tensor_add(acc, P(0), P(1))
                            j0 = 2
                        for j in range(j0, nblk):
                            off = off_of(j)
                            nc.vector.tensor_add(
                                acc[:, off:512], acc[:, off:512], P(j)[:, off:512]
                            )
                        # AV: outT = V^T P^T via lhsT=V natural
                        ps_o = ps_av.tile([128, 512], F32, tag="o")
                        for j in range(nblk):
                            off = off_of(j)
                            nc.tensor.matmul(
                                ps_o[:, off:512],
                                lhsT=vt[:, j, hh, :],
                                rhs=P(j)[:, off:512],
                                start=(j == 0),
                                stop=(j == nblk - 1),
                            )
                            yield
                        if pend[0] is not None:
                            yield from finish_pend()
                        pend[0] = (h, c, acc, ps_o)

                    def gen_p2_head(h):
                        for c in range(NT):
                            yield from gen_p2_chunk(h, c)

                    with (
                        tc.tile_pool(name=f"p1x_{_rep}", bufs=1) as p1x,
                        tc.tile_pool(name=f"p1w_{_rep}", bufs=2) as p1w,
                        tc.tile_pool(name=f"p1v_{_rep}", bufs=1) as p1v,
                    ):
                        # ---- prologue DMAs; consts ride after the first loads.
                        # w0 split so the warm matmuls start on the first chunk;
                        # wv rides the scalar DMA queue in parallel with xs.
                        w0 = p1w.tile([128, 2, CT, 128], F16, tag="w")
                        nc.sync.dma_start(w0[:, :, 0:2, :], wqk[:, 0, :, 0:2, :])
                        nc.sync.dma_start(w0[:, :, 2:, :], wqk[:, 0, :, 2:, :])
                        xs = p1x.tile([128, CT, T], F16)
                        for cc in range(CT):
                            nc.sync.dma_start(xs[:, cc, :], xT[:, cc, :])
                        if _rep == 0:
                            load_consts()
                        wv0 = p1v.tile([128, CT, 512], F16, tag="wv", name=f"wv0_{_rep}")
                        nc.scalar.dma_start(wv0, wv[:, :, 0, :])

                        # ---- warm start: head-0 q/k chunk-outer so PE works
                        # while the xs chunks stream in (7 psum groups; k0's
                        # last chunk follows in normal order)
                        q0 = qkp.tile([128, T], F16, tag="qk", name=f"q_{_rep}_0")
                        k0 = qkp.tile([128, T], F16, tag="qk", name=f"k_{_rep}_0")
                        qk_tiles[0] = (q0, k0)
                        warm = [(0, 0), (0, 1), (0, 2), (0, 3), (1, 0), (1, 1), (1, 2)]
                        wpool = [ps_p1, ps_p1, ps_sc, ps_sc, ps_sc, ps_av, ps_av]
                        wtag = ["p1", "p1", "s", "s", "s", "o", "o"]
                        wps = {
                            sc: wpool[i].tile(
                                [128, 512], F32, tag=wtag[i], name=f"warm_{_rep}_{i}"
                            )
                            for i, sc in enumerate(warm)
                        }
                        for cc in range(CT):
                            for s, c in warm:
                                nc.tensor.matmul(
                                    wps[(s, c)],
                                    lhsT=w0[:, s, cc, :],
                                    rhs=xs[:, cc, 512 * c : 512 * (c + 1)],
                                    start=(cc == 0),
                                    stop=(cc == CT - 1),
                                )
                        for s, c in warm:
                            dst = (q0, k0)[s]
                            nc.vector.tensor_scalar_add(
                                dst[:, 512 * c : 512 * (c + 1)],
                                wps[(s, c)],
                                bias_sb[:, s * HPC : s * HPC + 1],
                            )
                        # k0 chunk 3 (the one warm slot we didn't have)
                        ps = ps_p1.tile([128, 512], F32, tag="p1")
                        for cc in range(CT):
                            nc.tensor.matmul(
                                ps,
                                lhsT=w0[:, 1, cc, :],
                                rhs=xs[:, cc, 512 * 3 : 512 * 4],
                                start=(cc == 0),
                                stop=(cc == CT - 1),
                            )
                        nc.vector.tensor_scalar_add(
                            k0[:, 512 * 3 : 512 * 4], ps, bias_sb[:, HPC : HPC + 1]
                        )

                        # V for heads 0-3 (xs is fully resident by now)
                        for _ in gen_V(0, wv0, xs):
                            pass

                        # ---- pipeline: p1(h) [+ V grp 1 at h=4] ∥ p2(h-1)
                        for h in range(1, HPC):
                            w_sb = p1w.tile([128, 2, CT, 128], F16, tag="w")
                            nc.gpsimd.dma_start(w_sb, wqk[:, h])
                            pri = gen_p1_head(h, w_sb, xs)
                            n_pri = 128
                            if h == 4:
                                wv1 = p1v.tile(
                                    [128, CT, 512], F16, tag="wv", name=f"wv1_{_rep}"
                                )
                                nc.scalar.dma_start(wv1, wv[:, :, 1, :])
                                pri = itertools.chain(pri, gen_V(1, wv1, xs))
                                n_pri += TB * (CT + (1 if vbias else 0))
                            sec = gen_p2_head(h - 1)
                            _drive(pri, sec, n_pri / 84.0)

                    # ---- tail: p2(7) ∥ output projection waves (phase-1
                    # SBUF pools are closed; wproj + ot reload take the space)
                    with (
                        tc.tile_pool(name=f"wpp_{_rep}", bufs=16) as wpp,
                        tc.tile_pool(name=f"ysp_{_rep}", bufs=4) as ysp,
                        tc.tile_pool(name=f"otlp_{_rep}", bufs=1) as otlp,
                    ):
                        wp_tiles = []
                        for i in range(CT):
                            wp = wpp.tile(
                                [128, HPC, 128], F16, tag="wp", name=f"wp_{_rep}_{i}"
                            )
                            nc.sync.dma_start(wp, wproj[:, i])
                            wp_tiles.append(wp)
                        ot_loads = {}
                        for c in range(NT):
                            otl = otlp.tile(
                                [128, 4, 512], F16, tag=f"otl_{c}",
                                name=f"otl_{_rep}_{c}",
                            )
                            nc.sync.dma_start(otl, otd[:, c])
                            ot_loads[c] = otl

                        def gen_p3_wave(c):
                            otl = ot_loads[c]
                            for i in range(CT):
                                ps_y = ps_p1.tile([128, 512], F32, tag="p1")
                                for hh2 in range(HPC):
                                    rhs = (
                                        otl[:, hh2, :]
                                        if hh2 < 4
                                        else ot_res[(hh2, c)][:]
                                    )
                                    nc.tensor.matmul(
                                        ps_y,
                                        lhsT=wp_tiles[i][:, hh2, :],
                                        rhs=rhs,
                                        start=(hh2 == 0),
                                        stop=(hh2 == HPC - 1),
                                    )
                                    yield
                                ys = ysp.tile([128, 512], F16, tag="ys")
                                nc.vector.tensor_copy(ys, ps_y)
                                nc.sync.dma_start(
                                    yT_r[:, i, 512 * c : 512 * (c + 1)], ys
                                )

                        chunks = [gen_p2_chunk(7, c) for c in range(NT)]
                        for _ in chunks[0]:
                            pass
                        for _ in chunks[1]:
                            pass
                        _drive(gen_p3_wave(0), chunks[2], 128 / 26.0)
                        _drive(gen_p3_wave(1), chunks[3], 128 / 34.0)
                        for _ in finish_pend():
                            pass
                        for _ in gen_p3_wave(2):
                            pass
                        for _ in gen_p3_wave(3):
                            pass

    nc.compile()
    return nc


_NC_CACHE = {}


def _get_nc(vbias=True):
    if vbias not in _NC_CACHE:
        _NC_CACHE[vbias] = build_nc(vbias=vbias)
    return _NC_CACHE[vbias]


def _prep_inputs(x, Wqkv, bqkv, Wproj):
    """Host-side shard + pre-tile + fp16 cast. Returns list of 8 in_maps,
    core index = g * B + b."""
    x = np.asarray(x, dtype=np.float32)
    Wqkv = np.asarray(Wqkv, dtype=np.float32)
    Wproj = np.asarray(Wproj, dtype=np.float32)
    bqkv = np.asarray(bqkv, dtype=np.float32)

    p = np.arange(128)[:, None]
    f = np.arange(128)[None, :]
    tri = (f >= p).astype(np.float16)

    # xT tiles per batch: [128, CT, T] with [p, o, t] = x[b, t, o*128+p]
    xT_b = []
    for b in range(B):
        xt = x[b].T.astype(np.float16)  # [C, T]
        xT_b.append(np.ascontiguousarray(xt.reshape(CT, 128, T).transpose(1, 0, 2)))

    in_maps = [None] * (G * B)
    for g in range(G):
        lo = g * 1024
        # q/k weights: [128 p, 8 h, 2 s, 16 o, 128 m]
        wq = Wqkv[:, lo : lo + 1024]  # [C, 1024]
        wk = Wqkv[:, C + lo : C + lo + 1024]
        wqk_t = np.empty((128, HPC, 2, CT, 128), dtype=np.float16)
        for s, wmat in enumerate((wq, wk)):
            # wmat[(o p), (h m)] -> [p, h, o, m]
            wqk_t[:, :, s] = (
                wmat.reshape(CT, 128, HPC, 128).transpose(1, 2, 0, 3).astype(np.float16)
            )
        # v weights natural: [128 p(c), 16 o, 2 u, 512 m]
        wvg = Wqkv[:, 2 * C + lo : 2 * C + lo + 1024]  # [C, 1024]
        wv_t = np.ascontiguousarray(
            wvg.reshape(CT, 128, 2, 512).transpose(1, 0, 2, 3).astype(np.float16)
        )
        # biases
        bq = bqkv[lo : lo + 1024].reshape(HPC, 128).T  # [128, 8]
        bk = bqkv[C + lo : C + lo + 1024].reshape(HPC, 128).T
        bqk_t = np.ascontiguousarray(
            np.concatenate([bq, bk], axis=1).astype(np.float32)
        )  # [128, 16]
        bv_t = np.ascontiguousarray(
            bqkv[2 * C + lo : 2 * C + lo + 1024].reshape(1, 2, 512).astype(np.float16)
        )
        # proj weights: [128 p, 16 i, 8 h, 128 m]
        wpm = Wproj[lo : lo + 1024, :]  # [1024, C] = [(h p), (i m)]
        wp_t = np.ascontiguousarray(
            wpm.reshape(HPC, 128, CT, 128).transpose(1, 2, 0, 3).astype(np.float16)
        )
        for b in range(B):
            in_maps[g * B + b] = dict(
                xT=xT_b[b], wqk=wqk_t, wv=wv_t, wproj=wp_t,
                bqk=bqk_t, bv=bv_t, tri=tri,
            )
    return in_maps


def kernel(x, Wqkv, bqkv, Wproj, bproj):
    x = np.asarray(x)
    nc = _get_nc(vbias=bool(np.any(np.asarray(bqkv)[2 * C :])))
    in_maps = _prep_inputs(x, Wqkv, bqkv, Wproj)
    res = run_bass_kernel_spmd(nc, in_maps, core_ids=list(range(G * B)))
    y = np.empty((B, T, C), dtype=np.float32)
    bp = np.asarray(bproj, dtype=np.float32)
    for b in range(B):
        acc = res.results[b]["yT"].astype(np.float32)
        for g in range(1, G):
            acc = acc + res.results[g * B + b]["yT"].astype(np.float32)
        y[b] = acc.T + bp[None, :]
    return y
